# revision 1
# baseline (speedup 1.0000x reference)
"""Trainium2 Bass kernel for nn_Attn: out = softmax(v . (W @ q_s + b)) over s.

Key algebraic identity:
    energies[s] = v . (W @ q[s] + b) = q[s] . (W^T v) + (v . b)
The (v . b) term is constant across s and softmax is shift-invariant, so it
drops out. The kernel therefore computes u = W^T v (tiny), then a matvec
energies = question @ u, then a global softmax. This is memory-bound:
question (128 MiB fp32) must stream from HBM once; everything else is noise.

Distribution over 8 NeuronCores:
  - tokens (seq_len) sharded 8-way: core r handles tokens [r*4096, (r+1)*4096)
  - question is passed TRANSPOSED per shard ([H, 4096] contiguous) so the
    TensorEngine can contract over H (PE contracts over the partition dim)
  - W column-sharded: core r computes u[r*128:(r+1)*128]; AllGather(512 B) -> u
  - v replicated
  - local energies [4096] AllGathered (16 KB/rank); every core computes the
    global softmax and writes the full [32768] output; host takes core 0's.
"""

import numpy as np

S = 32768
H = 1024
NCORES = 8
SHARD = S // NCORES  # 4096 tokens per core
HC = H // 128  # 8 h-chunks of 128
ST = SHARD // 512  # 8 s-tiles of 512 tokens

_cached = {}


def _build():
    """Build + compile the SPMD Bass module (same NEFF on all 8 cores)."""
    from contextlib import ExitStack

    import concourse.bass as bass
    import concourse.mybir as mybir
    import concourse.tile as tile
    from concourse import bacc
    from concourse.masks import make_identity

    f32 = mybir.dt.float32
    AX = mybir.AxisListType
    ts = bass.ts

    nc = bacc.Bacc(
        "TRN2", target_bir_lowering=False, debug=False, num_devices=NCORES
    )

    qt = nc.dram_tensor("qt", [H, SHARD], f32, kind="ExternalInput")
    wc = nc.dram_tensor("wc", [H, 128], f32, kind="ExternalInput")
    vt = nc.dram_tensor("vt", [128, HC], f32, kind="ExternalInput")
    out = nc.dram_tensor("out", [S], f32, kind="ExternalOutput")

    rg = [list(range(NCORES))]

    with tile.TileContext(nc) as tc, ExitStack() as ctx:
        const = ctx.enter_context(tc.tile_pool(name="const", bufs=1))
        qpool = ctx.enter_context(tc.tile_pool(name="qpool", bufs=3))
        work = ctx.enter_context(tc.tile_pool(name="work", bufs=1))
        psum_e = ctx.enter_context(tc.tile_pool(name="psum_e", bufs=3, space="PSUM"))
        psum_s = ctx.enter_context(tc.tile_pool(name="psum_s", bufs=2, space="PSUM"))
        dram = ctx.enter_context(tc.tile_pool(name="dram", bufs=1, space="DRAM"))

        # ---- u = (W^T v) slice owned by this rank (128 entries) ----
        v_sb = const.tile([128, HC], f32)
        nc.sync.dma_start(v_sb[:], vt[:])
        wc_view = wc[:].rearrange("(c p) m -> c p m", p=128)
        pu = psum_s.tile([128, 1], f32, tag="stat")
        for c in range(HC):
            wc_sb = const.tile([128, 128], f32, tag=f"wc{c}")
            nc.sync.dma_start(wc_sb[:], wc_view[c])
            nc.tensor.matmul(
                pu[:], lhsT=wc_sb[:], rhs=v_sb[:, c : c + 1],
                start=(c == 0), stop=(c == HC - 1),
            )
        u_loc = work.tile([128, 1], f32)
        nc.vector.tensor_copy(u_loc[:], pu[:])
        u_loc_dram = dram.tile([128, 1], f32)
        nc.sync.dma_start(u_loc_dram[:], u_loc[:])
        u_all_dram = dram.tile([H, 1], f32)
        nc.gpsimd.collective_compute(
            "AllGather", mybir.AluOpType.bypass, replica_groups=rg,
            ins=[u_loc_dram.opt()], outs=[u_all_dram.opt()],
        )
        u_sb = const.tile([128, HC], f32)
        u_view = u_all_dram[:].rearrange("(c p) one -> c p one", p=128)
        for c in range(HC):
            nc.sync.dma_start(u_sb[:, c : c + 1], u_view[c])

        # ---- local energies: e[s] = sum_h u[h] * qt[h, s] ----
        e_loc = work.tile([1, SHARD], f32)
        qt_view = qt[:].rearrange("(c p) (t s) -> t p c s", p=128, s=512)
        for t in range(ST):
            q_sb = qpool.tile([128, HC, 512], f32, tag="q")
            nc.sync.dma_start(q_sb[:], qt_view[t])
            pe = psum_e.tile([1, 512], f32, tag="pe")
            for c in range(HC):
                nc.tensor.matmul(
                    pe[:], lhsT=u_sb[:, c : c + 1], rhs=q_sb[:, c, :],
                    start=(c == 0), stop=(c == HC - 1),
                )
            nc.scalar.copy(e_loc[:, ts(t, 512)], pe[:])

        e_loc_dram = dram.tile([1, SHARD], f32)
        nc.sync.dma_start(e_loc_dram[:], e_loc[:])
        e_all_dram = dram.tile([NCORES, SHARD], f32)
        nc.gpsimd.collective_compute(
            "AllGather", mybir.AluOpType.bypass, replica_groups=rg,
            ins=[e_loc_dram.opt()], outs=[e_all_dram.opt()],
        )

        # ---- global softmax over all 32768 energies ----
        e_all = work.tile([128, S // 128], f32)  # [128, 256], token = p*256+f
        nc.sync.dma_start(
            e_all[:], e_all_dram[:].rearrange("r (q f) -> (r q) f", f=S // 128)
        )
        rowmax = work.tile([128, 1], f32)
        nc.vector.reduce_max(rowmax[:], e_all[:], axis=AX.X)
        ident = const.tile([128, 128], f32)
        make_identity(nc, ident[:])
        ptr = psum_s.tile([1, 128], f32, tag="stat")
        nc.tensor.transpose(ptr[:], rowmax[:], ident[:])
        tmax = work.tile([1, 128], f32)
        nc.vector.tensor_copy(tmax[:], ptr[:])
        m = work.tile([1, 1], f32)
        nc.vector.reduce_max(m[:], tmax[:], axis=AX.X)
        negm = work.tile([1, 1], f32)
        nc.scalar.mul(negm[:], m[:], -1.0)

        ones_row = const.tile([1, 128], f32)
        nc.gpsimd.memset(ones_row[:], 1.0)
        ones_col = const.tile([128, 1], f32)
        nc.gpsimd.memset(ones_col[:], 1.0)

        pb = psum_s.tile([128, 1], f32, tag="stat")
        nc.tensor.matmul(pb[:], lhsT=ones_row[:], rhs=negm[:], start=True, stop=True)
        negm_b = work.tile([128, 1], f32)
        nc.vector.tensor_copy(negm_b[:], pb[:])

        ex = work.tile([128, S // 128], f32)
        rowsum = work.tile([128, 1], f32)
        nc.scalar.activation(
            ex[:], e_all[:], mybir.ActivationFunctionType.Exp,
            bias=negm_b[:], scale=1.0, accum_out=rowsum[:],
        )
        pts = psum_s.tile([1, 1], f32, tag="stat")
        nc.tensor.matmul(pts[:], lhsT=rowsum[:], rhs=ones_col[:], start=True, stop=True)
        stot = work.tile([1, 1], f32)
        nc.vector.tensor_copy(stot[:], pts[:])
        rtot = work.tile([1, 1], f32)
        nc.vector.reciprocal(rtot[:], stot[:])
        pbr = psum_s.tile([128, 1], f32, tag="stat")
        nc.tensor.matmul(pbr[:], lhsT=ones_row[:], rhs=rtot[:], start=True, stop=True)
        rtot_b = work.tile([128, 1], f32)
        nc.vector.tensor_copy(rtot_b[:], pbr[:])

        outt = work.tile([128, S // 128], f32)
        nc.scalar.mul(outt[:], ex[:], rtot_b[:])
        nc.sync.dma_start(out[:].rearrange("(p f) -> p f", f=S // 128), outt[:])

    nc.compile()
    return nc


def _get_nc():
    if "nc" not in _cached:
        _cached["nc"] = _build()
    return _cached["nc"]


def make_in_maps(question, W, v):
    q = np.ascontiguousarray(np.asarray(question, dtype=np.float32))
    Wn = np.ascontiguousarray(np.asarray(W, dtype=np.float32))
    vn = np.ascontiguousarray(np.asarray(v, dtype=np.float32))
    qt = q.T  # [H, S]
    vt = np.ascontiguousarray(vn.reshape(HC, 128).T)  # [128, HC]
    in_maps = []
    for r in range(NCORES):
        in_maps.append(
            {
                "qt": np.ascontiguousarray(qt[:, r * SHARD : (r + 1) * SHARD]),
                "wc": np.ascontiguousarray(Wn[:, r * 128 : (r + 1) * 128]),
                "vt": vt,
            }
        )
    return in_maps


def run(question, W, v, **spmd_kwargs):
    """Run the SPMD kernel; returns (out [S] fp32, BassKernelResults)."""
    from concourse.bass_utils import run_bass_kernel_spmd

    nc = _get_nc()
    in_maps = make_in_maps(question, W, v)
    res = run_bass_kernel_spmd(nc, in_maps, core_ids=list(range(NCORES)), **spmd_kwargs)
    return np.asarray(res.results[0]["out"], dtype=np.float32), res


def kernel(question, W, b, v):
    out, _ = run(question, W, v)
    return out.reshape(1, 1, S)
